# revision 13
# baseline (speedup 1.0000x reference)
"""BinaryLinear Trainium2 kernel: y = x @ sign(W).T + bias.

Contract: kernel(x, weight, bias) takes FULL unsharded numpy inputs
(x [32768,1024] f32, weight [1024,1024] f32, bias [1024] f32) and returns
the FULL output [32768,1024] f32.

Strategy (8 NeuronCores, data-parallel over tokens):
  - x is sharded into 8 x [4096, 1024] row shards; weight+bias replicated.
  - Per core, everything happens on-device:
      * weight prep, per 128-row block: DMA W fp32 (scalar ring), ACT Sign
        -> bf16 (exact for {-1,0,+1}), PE-transpose 128x128 blocks, DVE
        evicts to wT [d, o] bf16.
      * x streamed fp32 (sync ring), ACT-cast to bf16 per 128-token
        subtile, PE-transposed (16-bit rate) into a 1-bank PSUM tile, ACT
        evicts to xT [d, ic, tok] bf16. bf16 inputs give ~2.4e-3 rel err
        vs the 2e-2 budget.
      * matmuls run in NORMAL bf16 mode: the PE streams one column/cycle
        regardless of dtype (fp8 DoubleRow only repacks the stationary
        operand; measured no faster end-to-end), so a single bf16 pass
        (512 matmuls x 512 cols ~ 109us) is the PE floor and keeps
        ACT/DVE load low.
      * per 128-token subtile: 16 matmuls accumulate a [128, 2x512] PSUM
        tile; DVE adds the broadcast bias on eviction, writing y in fp16
        (halves output DMA; host upcasts to fp32).
  - HAM keeps the PE at 1.2 GHz until it sees ~3.4us of sustained work and
    drops it back after any ~3.4us idle window, so the whole schedule is
    built to keep the PE queue non-empty: serialized dummy matmuls bridge
    the startup DMA wait, weight/x/y traffic ride three separate DMA
    rings, and emission is software-pipelined one macro-tile deep with x
    DMA a further macro ahead.
"""

import numpy as np

import concourse.bass as bass  # noqa: F401  (bass types used via bacc)
import concourse.mybir as mybir
import concourse.tile as tile
from concourse import bacc
from concourse.bass_utils import run_bass_kernel_spmd
from concourse.masks import make_identity

P = 128
N_CORES = 8
F32 = mybir.dt.float32
F16 = mybir.dt.float16
BF16 = mybir.dt.bfloat16


def build_kernel(ntok: int, d: int, o: int, macro: int = 512, warm: int = 12):
    """Build the per-core Bass program for x [ntok, d] f32 -> y [ntok, o] f16."""
    assert ntok % macro == 0 and macro % P == 0 and d % P == 0 and o % P == 0
    NS = macro // P  # token subtiles per macro tile
    NM = ntok // macro  # macro tiles
    IC = d // P  # contraction 128-chunks
    OC = o // P  # output-feature 128-blocks
    OGW = min(512, o)  # psum bank free width
    NOG = o // OGW  # output groups
    WG = min(4, OC)  # weight blocks per prep group

    nc = bacc.Bacc(None, target_bir_lowering=False)

    x = nc.dram_tensor("x", [ntok, d], F32, kind="ExternalInput")
    w = nc.dram_tensor("w", [o, d], F32, kind="ExternalInput")
    bias = nc.dram_tensor("bias", [1, o], F32, kind="ExternalInput")
    y = nc.dram_tensor("y", [ntok, o], F16, kind="ExternalOutput")

    xr = x[:].rearrange("(m s p) d -> p m s d", p=P, s=NS)
    yr = y[:].rearrange("(m s p) o -> p m s o", p=P, s=NS)
    wr = w[:].rearrange("(oc p) d -> p oc d", p=P)

    with tile.TileContext(nc) as tc:
        with (
            tc.tile_pool(name="const", bufs=1) as const,
            tc.tile_pool(name="wstage", bufs=1) as wstage,
            tc.tile_pool(name="xpool", bufs=2) as xpool,
            tc.tile_pool(name="xhpool", bufs=2) as xhpool,
            tc.tile_pool(name="xtpool", bufs=2 * NS) as xtpool,
            tc.tile_pool(name="ypool", bufs=4) as ypool,
            tc.tile_pool(name="tpsum", bufs=2, space="PSUM") as tpsum,
            tc.tile_pool(name="ypsum", bufs=3, space="PSUM") as ypsum,
        ):
            # ---- x DMA on the sync ring, issued a macro ahead ----
            def xdma(m):
                t = xpool.tile([P, NS, d], F32, tag="x_sb", name=f"x{m % 2}")
                h = NS // 2
                nc.sync.dma_start(t[:, :h], xr[:, m, :h])
                nc.sync.dma_start(t[:, h:], xr[:, m, h:])
                return t

            x_tiles = {0: xdma(0)}

            # identity first so the gpsimd sequencer builds it before its
            # share of the weight-DMA instructions below
            ident = const.tile([P, P], BF16, name="ident")
            make_identity(nc, ident)

            # ---- weight DMA per 128-row block, split across the scalar and
            # gpsimd rings: one ring sustains only ~100 GB/s, so 4 MB on a
            # single ring (~22us) would gate the whole sign->transpose chain
            w_sb = wstage.tile([P, OC, d], F32)
            for j in range(OC):
                dma = nc.scalar if j % 2 == 0 else nc.gpsimd
                dma.dma_start(w_sb[:, j], wr[:, j])
            x_tiles[1] = xdma(1)

            bias_bc = const.tile([P, o], F32)
            nc.scalar.dma_start(bias_bc[:], bias[:].to_broadcast((P, o)))

            # HAM warm-up: serialized dummy matmuls (same PSUM target) bridge
            # the startup DMA wait so the PE is at 2.4 GHz when real work
            # lands. Memset on DVE so the chain starts immediately.
            dummy = const.tile([P, 512], BF16, name="dummy")
            nc.vector.memset(dummy[:], 0.0)
            dpsum = ypsum.tile([P, 2, OGW], F32, tag="yp", name="ypdummy")
            for _ in range(warm):
                nc.tensor.matmul(
                    dpsum[:, 0], dummy[:, :P], dummy[:], start=True, stop=True
                )

            # ---- weight prep: sign (ACT), PE transpose, evict (DVE) ----
            ws = wstage.tile([P, OC, d], BF16, name="ws")
            wT = const.tile([P, IC, o], BF16, name="wT")

            def weight_prep(g):
                for j in range(g, g + WG):
                    nc.scalar.sign(ws[:, j], w_sb[:, j])
                    pw = tpsum.tile([P, IC, P], BF16, tag="pt", name="pw")
                    for ic in range(IC):
                        nc.tensor.transpose(
                            pw[:, ic], ws[:, j, ic * P : (ic + 1) * P], ident[:]
                        )
                    nc.vector.tensor_copy(wT[:, :, j * P : (j + 1) * P], pw[:])

            # ---- per-macro sections ----
            def split_compute(m, x_sb):
                xh = xhpool.tile([P, NS, d], BF16, tag="xh")
                cast = nc.scalar.copy
                xts = []
                for s in range(NS):
                    cast(xh[:, s], x_sb[:, s])
                    pt = tpsum.tile([P, IC, P], BF16, tag="pt")
                    for ic in range(IC):
                        nc.tensor.transpose(
                            pt[:, ic], xh[:, s, ic * P : (ic + 1) * P], ident[:]
                        )
                    xt = xtpool.tile([P, IC, P], BF16, tag="xT", name=f"xT{s}")
                    cast(xt[:], pt[:])
                    xts.append(xt)
                return xts

            def mm_tile(yp, xt, osl, lead):
                for ic in range(IC):
                    nc.tensor.matmul(
                        yp,
                        xt[:, ic],
                        wT[:, ic, osl],
                        start=(lead and ic == 0),
                        stop=(ic == IC - 1),
                    )

            def mm_section(m, xts):
                for s in range(NS):
                    yp2 = ypsum.tile([P, 2, OGW], F32, tag="yp")
                    for og in range(NOG):
                        osl = slice(og * OGW, (og + 1) * OGW)
                        mm_tile(yp2[:, og], xts[s], osl, lead=True)
                    y_sb = ypool.tile([P, o], F16, tag="y_sb")
                    nc.vector.tensor_tensor(
                        y_sb[:], yp2[:], bias_bc[:], mybir.AluOpType.add
                    )
                    nc.scalar.dma_start(yr[:, m, s], y_sb[:])

            def mm_first(xts):
                # macro 0, og-major: og=0 matmuls need only the first weight
                # group's prep; the second group's prep slots between the og
                # passes (its DMA long done), off the startup critical path.
                ysb = {
                    s: ypool.tile([P, o], F16, tag="y_sb", name=f"ysbf{s}")
                    for s in range(NS)
                }
                for og in range(NOG):
                    if og >= 1 and og * WG < OC:
                        weight_prep(og * WG)
                    osl = slice(og * OGW, (og + 1) * OGW)
                    for s in range(NS):
                        ypf = ypsum.tile(
                            [P, 2, OGW], F32, tag="yp", name=f"ypf{s % 2}"
                        )
                        mm_tile(ypf[:, 0], xts[s], osl, lead=True)
                        nc.vector.tensor_tensor(
                            ysb[s][:, osl],
                            ypf[:, 0],
                            bias_bc[:, osl],
                            mybir.AluOpType.add,
                        )
                for g in range(max(1, NOG) * WG, OC, WG):
                    weight_prep(g)
                for s in range(NS):
                    nc.scalar.dma_start(yr[:, 0, s], ysb[s][:])

            # ---- main loop, software-pipelined one macro deep ----
            weight_prep(0)
            xts0 = split_compute(0, x_tiles[0])
            mm_first(xts0)
            prev = None
            for m in range(1, NM):
                if m + 1 < NM:
                    x_tiles[m + 1] = xdma(m + 1)
                cur = split_compute(m, x_tiles[m])
                if m >= 2:
                    mm_section(m - 1, prev)
                prev = cur
            mm_section(NM - 1, prev)

    nc.compile()
    return nc


_NC_CACHE: dict = {}


def _get_nc(ntok, d, o):
    key = (ntok, d, o)
    if key not in _NC_CACHE:
        _NC_CACHE[key] = build_kernel(ntok, d, o)
    return _NC_CACHE[key]


def kernel(x, weight, bias):
    x = np.ascontiguousarray(np.asarray(x, dtype=np.float32))
    weight = np.ascontiguousarray(np.asarray(weight, dtype=np.float32))
    bias = np.ascontiguousarray(np.asarray(bias, dtype=np.float32))
    ntok, d = x.shape
    o = weight.shape[0]
    assert ntok % N_CORES == 0
    shard = ntok // N_CORES

    nc = _get_nc(shard, d, o)
    bias2d = bias.reshape(1, o)
    in_maps = [
        {"x": x[i * shard : (i + 1) * shard], "w": weight, "bias": bias2d}
        for i in range(N_CORES)
    ]
    res = run_bass_kernel_spmd(nc, in_maps, core_ids=list(range(N_CORES)))
    out = np.concatenate([np.asarray(r["y"]) for r in res.results], axis=0)
    return out.astype(np.float32)


# revision 15
# speedup vs baseline: 1.0107x; 1.0107x over previous
"""BinaryLinear Trainium2 kernel: y = x @ sign(W).T + bias.

Contract: kernel(x, weight, bias) takes FULL unsharded numpy inputs
(x [32768,1024] f32, weight [1024,1024] f32, bias [1024] f32) and returns
the FULL output [32768,1024] f32.

Strategy (8 NeuronCores, data-parallel over tokens):
  - x is sharded into 8 x [4096, 1024] row shards; weight+bias replicated.
  - Per core, everything happens on-device:
      * weight prep, per 128-row block: DMA W fp32 (scalar ring), ACT Sign
        -> bf16 (exact for {-1,0,+1}), PE-transpose 128x128 blocks, DVE
        evicts to wT [d, o] bf16.
      * x streamed fp32 (sync ring), ACT-cast to bf16 per 128-token
        subtile, PE-transposed (16-bit rate) into a 1-bank PSUM tile, ACT
        evicts to xT [d, ic, tok] bf16. bf16 inputs give ~2.4e-3 rel err
        vs the 2e-2 budget.
      * matmuls run in NORMAL bf16 mode: the PE streams one column/cycle
        regardless of dtype (fp8 DoubleRow only repacks the stationary
        operand; measured no faster end-to-end), so a single bf16 pass
        (512 matmuls x 512 cols ~ 109us) is the PE floor and keeps
        ACT/DVE load low.
      * per 128-token subtile: 16 matmuls accumulate a [128, 2x512] PSUM
        tile; DVE adds the broadcast bias on eviction, writing y in fp16
        (halves output DMA; host upcasts to fp32).
  - HAM keeps the PE at 1.2 GHz until it sees ~3.4us of sustained work and
    drops it back after any ~3.4us idle window, so the whole schedule is
    built to keep the PE queue non-empty: serialized dummy matmuls bridge
    the startup DMA wait, weight/x/y traffic ride three separate DMA
    rings, and emission is software-pipelined one macro-tile deep with x
    DMA a further macro ahead.
"""

import numpy as np

import concourse.bass as bass  # noqa: F401  (bass types used via bacc)
import concourse.mybir as mybir
import concourse.tile as tile
from concourse import bacc
from concourse.bass_utils import run_bass_kernel_spmd
from concourse.masks import make_identity

P = 128
N_CORES = 8
F32 = mybir.dt.float32
F16 = mybir.dt.float16
BF16 = mybir.dt.bfloat16


def build_kernel(ntok: int, d: int, o: int, macro: int = 512, warm: int = 12):
    """Build the per-core Bass program for x [ntok, d] f32 -> y [ntok, o] f16."""
    assert ntok % macro == 0 and macro % P == 0 and d % P == 0 and o % P == 0
    NS = macro // P  # token subtiles per macro tile
    NM = ntok // macro  # macro tiles
    IC = d // P  # contraction 128-chunks
    OC = o // P  # output-feature 128-blocks
    OGW = min(512, o)  # psum bank free width
    NOG = o // OGW  # output groups
    WG = min(4, OC)  # weight blocks per prep group

    nc = bacc.Bacc(None, target_bir_lowering=False)

    x = nc.dram_tensor("x", [ntok, d], F32, kind="ExternalInput")
    w = nc.dram_tensor("w", [o, d], F32, kind="ExternalInput")
    bias = nc.dram_tensor("bias", [1, o], F32, kind="ExternalInput")
    y = nc.dram_tensor("y", [ntok, o], F16, kind="ExternalOutput")

    xr = x[:].rearrange("(m s p) d -> p m s d", p=P, s=NS)
    yr = y[:].rearrange("(m s p) o -> p m s o", p=P, s=NS)
    wr = w[:].rearrange("(oc p) d -> p oc d", p=P)

    with tile.TileContext(nc) as tc:
        with (
            tc.tile_pool(name="const", bufs=1) as const,
            tc.tile_pool(name="wstage", bufs=1) as wstage,
            tc.tile_pool(name="xpool", bufs=2) as xpool,
            tc.tile_pool(name="xhpool", bufs=2) as xhpool,
            tc.tile_pool(name="xtpool", bufs=2 * NS) as xtpool,
            tc.tile_pool(name="ypool", bufs=4) as ypool,
            tc.tile_pool(name="tpsum", bufs=2, space="PSUM") as tpsum,
            tc.tile_pool(name="ypsum", bufs=3, space="PSUM") as ypsum,
        ):
            # ---- x DMA on the sync ring, issued a macro ahead ----
            def xdma(m):
                t = xpool.tile([P, NS, d], F32, tag="x_sb", name=f"x{m % 2}")
                h = NS // 2
                nc.sync.dma_start(t[:, :h], xr[:, m, :h])
                nc.sync.dma_start(t[:, h:], xr[:, m, h:])
                return t

            x_tiles = {0: xdma(0)}

            # identity first so the gpsimd sequencer builds it before its
            # share of the weight-DMA instructions below
            ident = const.tile([P, P], BF16, name="ident")
            make_identity(nc, ident)

            # ---- weight DMA per 128-row block, split across the scalar and
            # gpsimd rings: one ring sustains only ~100 GB/s, so 4 MB on a
            # single ring (~22us) would gate the whole sign->transpose chain
            w_sb = wstage.tile([P, OC, d], F32)
            for j in range(OC):
                dma = nc.scalar if j % 2 == 0 else nc.gpsimd
                dma.dma_start(w_sb[:, j], wr[:, j])

            bias_bc = const.tile([P, o], F32)
            nc.scalar.dma_start(bias_bc[:], bias[:].to_broadcast((P, o)))

            # x1 prefetch AFTER the weight DMAs: the DMA engines serve
            # earlier-queued transfers first, and x1 isn't consumed until
            # ~35us while the weight bytes gate the whole startup chain
            x_tiles[1] = xdma(1)

            # HAM warm-up: serialized dummy matmuls (same PSUM target) bridge
            # the startup DMA wait so the PE is at 2.4 GHz when real work
            # lands. Memset on DVE so the chain starts immediately.
            dummy = const.tile([P, 512], BF16, name="dummy")
            nc.vector.memset(dummy[:], 0.0)
            dpsum = ypsum.tile([P, 2, OGW], F32, tag="yp", name="ypdummy")
            for _ in range(warm):
                nc.tensor.matmul(
                    dpsum[:, 0], dummy[:, :P], dummy[:], start=True, stop=True
                )

            # ---- weight prep: sign (ACT), PE transpose, evict (DVE) ----
            ws = wstage.tile([P, OC, d], BF16, name="ws")
            wT = const.tile([P, IC, o], BF16, name="wT")

            def weight_prep(g):
                for j in range(g, g + WG):
                    nc.scalar.sign(ws[:, j], w_sb[:, j])
                    pw = tpsum.tile([P, IC, P], BF16, tag="pt", name="pw")
                    for ic in range(IC):
                        nc.tensor.transpose(
                            pw[:, ic], ws[:, j, ic * P : (ic + 1) * P], ident[:]
                        )
                    nc.vector.tensor_copy(wT[:, :, j * P : (j + 1) * P], pw[:])

            # ---- per-macro sections ----
            def split_compute(m, x_sb):
                xh = xhpool.tile([P, NS, d], BF16, tag="xh")
                cast = nc.scalar.copy
                xts = []
                for s in range(NS):
                    cast(xh[:, s], x_sb[:, s])
                    pt = tpsum.tile([P, IC, P], BF16, tag="pt")
                    for ic in range(IC):
                        nc.tensor.transpose(
                            pt[:, ic], xh[:, s, ic * P : (ic + 1) * P], ident[:]
                        )
                    xt = xtpool.tile([P, IC, P], BF16, tag="xT", name=f"xT{s}")
                    cast(xt[:], pt[:])
                    xts.append(xt)
                return xts

            def mm_tile(yp, xt, osl, lead):
                for ic in range(IC):
                    nc.tensor.matmul(
                        yp,
                        xt[:, ic],
                        wT[:, ic, osl],
                        start=(lead and ic == 0),
                        stop=(ic == IC - 1),
                    )

            def mm_section(m, xts):
                for s in range(NS):
                    yp2 = ypsum.tile([P, 2, OGW], F32, tag="yp")
                    for og in range(NOG):
                        osl = slice(og * OGW, (og + 1) * OGW)
                        mm_tile(yp2[:, og], xts[s], osl, lead=True)
                    y_sb = ypool.tile([P, o], F16, tag="y_sb")
                    nc.vector.tensor_tensor(
                        y_sb[:], yp2[:], bias_bc[:], mybir.AluOpType.add
                    )
                    nc.scalar.dma_start(yr[:, m, s], y_sb[:])

            def mm_first(xts):
                # macro 0, og-major: og=0 matmuls need only the first weight
                # group's prep; the second group's prep slots between the og
                # passes (its DMA long done), off the startup critical path.
                ysb = {
                    s: ypool.tile([P, o], F16, tag="y_sb", name=f"ysbf{s}")
                    for s in range(NS)
                }
                for og in range(NOG):
                    if og >= 1 and og * WG < OC:
                        weight_prep(og * WG)
                    osl = slice(og * OGW, (og + 1) * OGW)
                    for s in range(NS):
                        ypf = ypsum.tile(
                            [P, 2, OGW], F32, tag="yp", name=f"ypf{s % 2}"
                        )
                        mm_tile(ypf[:, 0], xts[s], osl, lead=True)
                        nc.vector.tensor_tensor(
                            ysb[s][:, osl],
                            ypf[:, 0],
                            bias_bc[:, osl],
                            mybir.AluOpType.add,
                        )
                for g in range(max(1, NOG) * WG, OC, WG):
                    weight_prep(g)
                for s in range(NS):
                    nc.scalar.dma_start(yr[:, 0, s], ysb[s][:])

            # ---- main loop, software-pipelined one macro deep ----
            weight_prep(0)
            xts0 = split_compute(0, x_tiles[0])
            mm_first(xts0)
            prev = None
            for m in range(1, NM):
                cur = split_compute(m, x_tiles[m])
                if m + 1 < NM:
                    x_tiles[m + 1] = xdma(m + 1)
                if m >= 2:
                    mm_section(m - 1, prev)
                prev = cur
            mm_section(NM - 1, prev)

    nc.compile()
    return nc


_NC_CACHE: dict = {}


def _get_nc(ntok, d, o):
    key = (ntok, d, o)
    if key not in _NC_CACHE:
        _NC_CACHE[key] = build_kernel(ntok, d, o)
    return _NC_CACHE[key]


def kernel(x, weight, bias):
    x = np.ascontiguousarray(np.asarray(x, dtype=np.float32))
    weight = np.ascontiguousarray(np.asarray(weight, dtype=np.float32))
    bias = np.ascontiguousarray(np.asarray(bias, dtype=np.float32))
    ntok, d = x.shape
    o = weight.shape[0]
    assert ntok % N_CORES == 0
    shard = ntok // N_CORES

    nc = _get_nc(shard, d, o)
    bias2d = bias.reshape(1, o)
    in_maps = [
        {"x": x[i * shard : (i + 1) * shard], "w": weight, "bias": bias2d}
        for i in range(N_CORES)
    ]
    res = run_bass_kernel_spmd(nc, in_maps, core_ids=list(range(N_CORES)))
    out = np.concatenate([np.asarray(r["y"]) for r in res.results], axis=0)
    return out.astype(np.float32)


# revision 16
# speedup vs baseline: 1.0128x; 1.0021x over previous
"""BinaryLinear Trainium2 kernel: y = x @ sign(W).T + bias.

Contract: kernel(x, weight, bias) takes FULL unsharded numpy inputs
(x [32768,1024] f32, weight [1024,1024] f32, bias [1024] f32) and returns
the FULL output [32768,1024] f32.

Strategy (8 NeuronCores, data-parallel over tokens):
  - x is sharded into 8 x [4096, 1024] row shards; weight+bias replicated.
  - Per core, everything happens on-device:
      * weight prep, per 128-row block: DMA W fp32 (scalar ring), ACT Sign
        -> bf16 (exact for {-1,0,+1}), PE-transpose 128x128 blocks, DVE
        evicts to wT [d, o] bf16.
      * x streamed fp32 (sync ring), ACT-cast to bf16 per 128-token
        subtile, PE-transposed (16-bit rate) into a 1-bank PSUM tile, ACT
        evicts to xT [d, ic, tok] bf16. bf16 inputs give ~2.4e-3 rel err
        vs the 2e-2 budget.
      * matmuls run in NORMAL bf16 mode: the PE streams one column/cycle
        regardless of dtype (fp8 DoubleRow only repacks the stationary
        operand; measured no faster end-to-end), so a single bf16 pass
        (512 matmuls x 512 cols ~ 109us) is the PE floor and keeps
        ACT/DVE load low.
      * per 128-token subtile: 16 matmuls accumulate a [128, 2x512] PSUM
        tile; DVE adds the broadcast bias on eviction, writing y in fp16
        (halves output DMA; host upcasts to fp32).
  - HAM keeps the PE at 1.2 GHz until it sees ~3.4us of sustained work and
    drops it back after any ~3.4us idle window, so the whole schedule is
    built to keep the PE queue non-empty: serialized dummy matmuls bridge
    the startup DMA wait, weight/x/y traffic ride three separate DMA
    rings, and emission is software-pipelined one macro-tile deep with x
    DMA a further macro ahead.
"""

import numpy as np

import concourse.bass as bass  # noqa: F401  (bass types used via bacc)
import concourse.mybir as mybir
import concourse.tile as tile
from concourse import bacc
from concourse.bass_utils import run_bass_kernel_spmd
from concourse.masks import make_identity

P = 128
N_CORES = 8
F32 = mybir.dt.float32
F16 = mybir.dt.float16
BF16 = mybir.dt.bfloat16


def build_kernel(ntok: int, d: int, o: int, macro: int = 512, warm: int = 16):
    """Build the per-core Bass program for x [ntok, d] f32 -> y [ntok, o] f16."""
    assert ntok % macro == 0 and macro % P == 0 and d % P == 0 and o % P == 0
    NS = macro // P  # token subtiles per macro tile
    NM = ntok // macro  # macro tiles
    IC = d // P  # contraction 128-chunks
    OC = o // P  # output-feature 128-blocks
    OGW = min(512, o)  # psum bank free width
    NOG = o // OGW  # output groups
    WG = min(4, OC)  # weight blocks per prep group

    nc = bacc.Bacc(None, target_bir_lowering=False)

    x = nc.dram_tensor("x", [ntok, d], F32, kind="ExternalInput")
    w = nc.dram_tensor("w", [o, d], F32, kind="ExternalInput")
    bias = nc.dram_tensor("bias", [1, o], F32, kind="ExternalInput")
    y = nc.dram_tensor("y", [ntok, o], F16, kind="ExternalOutput")

    xr = x[:].rearrange("(m s p) d -> p m s d", p=P, s=NS)
    yr = y[:].rearrange("(m s p) o -> p m s o", p=P, s=NS)
    wr = w[:].rearrange("(oc p) d -> p oc d", p=P)

    with tile.TileContext(nc) as tc:
        with (
            tc.tile_pool(name="const", bufs=1) as const,
            tc.tile_pool(name="wstage", bufs=1) as wstage,
            tc.tile_pool(name="xpool", bufs=2) as xpool,
            tc.tile_pool(name="xhpool", bufs=2) as xhpool,
            tc.tile_pool(name="xtpool", bufs=2 * NS) as xtpool,
            tc.tile_pool(name="ypool", bufs=4) as ypool,
            tc.tile_pool(name="tpsum", bufs=2, space="PSUM") as tpsum,
            tc.tile_pool(name="ypsum", bufs=3, space="PSUM") as ypsum,
        ):
            # ---- x DMA on the sync ring, issued a macro ahead ----
            def xdma(m):
                t = xpool.tile([P, NS, d], F32, tag="x_sb", name=f"x{m % 2}")
                h = NS // 2
                nc.sync.dma_start(t[:, :h], xr[:, m, :h])
                nc.sync.dma_start(t[:, h:], xr[:, m, h:])
                return t

            x_tiles = {0: xdma(0)}

            # identity first so the gpsimd sequencer builds it before its
            # share of the weight-DMA instructions below
            ident = const.tile([P, P], BF16, name="ident")
            make_identity(nc, ident)

            # ---- weight DMA per 128-row block, split across the scalar and
            # gpsimd rings: one ring sustains only ~100 GB/s, so 4 MB on a
            # single ring (~22us) would gate the whole sign->transpose chain
            w_sb = wstage.tile([P, OC, d], F32)
            for j in range(OC):
                dma = nc.scalar if j % 2 == 0 else nc.gpsimd
                dma.dma_start(w_sb[:, j], wr[:, j])

            bias_bc = const.tile([P, o], F32)
            nc.scalar.dma_start(bias_bc[:], bias[:].to_broadcast((P, o)))

            # x1 prefetch AFTER the weight DMAs: the DMA engines serve
            # earlier-queued transfers first, and x1 isn't consumed until
            # ~35us while the weight bytes gate the whole startup chain
            x_tiles[1] = xdma(1)

            # HAM warm-up: serialized dummy matmuls (same PSUM target) bridge
            # the startup DMA wait so the PE is at 2.4 GHz when real work
            # lands. Memset on DVE so the chain starts immediately.
            dummy = const.tile([P, 512], BF16, name="dummy")
            nc.vector.memset(dummy[:], 0.0)
            dpsum = ypsum.tile([P, 2, OGW], F32, tag="yp", name="ypdummy")
            for _ in range(warm):
                nc.tensor.matmul(
                    dpsum[:, 0], dummy[:, :P], dummy[:], start=True, stop=True
                )

            # ---- weight prep: sign (ACT), PE transpose, evict (DVE) ----
            ws = wstage.tile([P, OC, d], BF16, name="ws")
            wT = const.tile([P, IC, o], BF16, name="wT")

            def weight_prep(g):
                for j in range(g, g + WG):
                    nc.scalar.sign(ws[:, j], w_sb[:, j])
                    pw = tpsum.tile([P, IC, P], BF16, tag="pt", name="pw")
                    for ic in range(IC):
                        nc.tensor.transpose(
                            pw[:, ic], ws[:, j, ic * P : (ic + 1) * P], ident[:]
                        )
                    nc.vector.tensor_copy(wT[:, :, j * P : (j + 1) * P], pw[:])

            # ---- per-macro sections ----
            def split_compute(m, x_sb):
                xh = xhpool.tile([P, NS, d], BF16, tag="xh")
                cast = nc.scalar.copy
                xts = []
                for s in range(NS):
                    cast(xh[:, s], x_sb[:, s])
                    pt = tpsum.tile([P, IC, P], BF16, tag="pt")
                    for ic in range(IC):
                        nc.tensor.transpose(
                            pt[:, ic], xh[:, s, ic * P : (ic + 1) * P], ident[:]
                        )
                    xt = xtpool.tile([P, IC, P], BF16, tag="xT", name=f"xT{s}")
                    cast(xt[:], pt[:])
                    xts.append(xt)
                return xts

            def mm_tile(yp, xt, osl, lead):
                for ic in range(IC):
                    nc.tensor.matmul(
                        yp,
                        xt[:, ic],
                        wT[:, ic, osl],
                        start=(lead and ic == 0),
                        stop=(ic == IC - 1),
                    )

            def mm_section(m, xts):
                for s in range(NS):
                    yp2 = ypsum.tile([P, 2, OGW], F32, tag="yp")
                    for og in range(NOG):
                        osl = slice(og * OGW, (og + 1) * OGW)
                        mm_tile(yp2[:, og], xts[s], osl, lead=True)
                    y_sb = ypool.tile([P, o], F16, tag="y_sb")
                    nc.vector.tensor_tensor(
                        y_sb[:], yp2[:], bias_bc[:], mybir.AluOpType.add
                    )
                    nc.scalar.dma_start(yr[:, m, s], y_sb[:])

            def mm_first(xts):
                # macro 0, og-major: og=0 matmuls need only the first weight
                # group's prep; the second group's prep slots between the og
                # passes (its DMA long done), off the startup critical path.
                ysb = {
                    s: ypool.tile([P, o], F16, tag="y_sb", name=f"ysbf{s}")
                    for s in range(NS)
                }
                for og in range(NOG):
                    if og >= 1 and og * WG < OC:
                        weight_prep(og * WG)
                    osl = slice(og * OGW, (og + 1) * OGW)
                    for s in range(NS):
                        ypf = ypsum.tile(
                            [P, 2, OGW], F32, tag="yp", name=f"ypf{s % 2}"
                        )
                        mm_tile(ypf[:, 0], xts[s], osl, lead=True)
                        nc.vector.tensor_tensor(
                            ysb[s][:, osl],
                            ypf[:, 0],
                            bias_bc[:, osl],
                            mybir.AluOpType.add,
                        )
                for g in range(max(1, NOG) * WG, OC, WG):
                    weight_prep(g)
                for s in range(NS):
                    nc.scalar.dma_start(yr[:, 0, s], ysb[s][:])

            # ---- main loop, software-pipelined one macro deep ----
            weight_prep(0)
            xts0 = split_compute(0, x_tiles[0])
            mm_first(xts0)
            prev = None
            for m in range(1, NM):
                cur = split_compute(m, x_tiles[m])
                if m + 1 < NM:
                    x_tiles[m + 1] = xdma(m + 1)
                if m >= 2:
                    mm_section(m - 1, prev)
                prev = cur
            mm_section(NM - 1, prev)

    nc.compile()
    return nc


_NC_CACHE: dict = {}


def _get_nc(ntok, d, o):
    key = (ntok, d, o)
    if key not in _NC_CACHE:
        _NC_CACHE[key] = build_kernel(ntok, d, o)
    return _NC_CACHE[key]


def kernel(x, weight, bias):
    x = np.ascontiguousarray(np.asarray(x, dtype=np.float32))
    weight = np.ascontiguousarray(np.asarray(weight, dtype=np.float32))
    bias = np.ascontiguousarray(np.asarray(bias, dtype=np.float32))
    ntok, d = x.shape
    o = weight.shape[0]
    assert ntok % N_CORES == 0
    shard = ntok // N_CORES

    nc = _get_nc(shard, d, o)
    bias2d = bias.reshape(1, o)
    in_maps = [
        {"x": x[i * shard : (i + 1) * shard], "w": weight, "bias": bias2d}
        for i in range(N_CORES)
    ]
    res = run_bass_kernel_spmd(nc, in_maps, core_ids=list(range(N_CORES)))
    out = np.concatenate([np.asarray(r["y"]) for r in res.results], axis=0)
    return out.astype(np.float32)
